# revision 21
# baseline (speedup 1.0000x reference)
"""Trainium2 Bass kernel for multi-head attention (B=2, S=2048, D=1024, H=16, HD=64).

Sharding: batch x head-quad. Core c owns batch c//4 and heads 4*(c%4)..4*(c%4)+3,
organized as two head PAIRS (p=0,1), each pair's two heads on partition halves
0:64 / 64:128 so the K=64 score matmuls run concurrently via PE row-group tiling.

Emission is a 64-slot conductor (slot t -> (pair, query-block ib, key-group g)):
  - ScalarE: 2 exp ACTIVATEs per slot (one per key tile, N=1024 covering BOTH
    heads side by side so the head A/B score matmuls share one PSUM tile and
    become ready together -> the PE runs them concurrently via row groups).
  - PE: scores for slot t+1 (PSUM ping-pong against the ACT reads), the attended
    matmuls of slot t-4 ([1|pad63|v] stationary emits softmax denominators on
    PSUM partition 0 for free), then deadline-scheduled filler (qkv projection
    blocks, output projection) against a running ns budget. Hard data-order
    requirements are enforced by force-running prerequisite units.
  - v transposes go through the DMA XBAR (SBUF->SBUF, transpose=True), not the PE.
  - last ib's attended is compressed into slots 60-63 (2 groups/slot) and its
    output projection uses wide PSUM tiles so the tail stays short.
Host packs weights into exact SBUF layouts; outputs are written bf16 and summed
on the host.
"""

import numpy as np
import ml_dtypes

import concourse.bacc as bacc
import concourse.tile as tile
import concourse.mybir as mybir
from concourse.bass_utils import run_bass_kernel_spmd
from concourse.masks import make_identity

B, S, D = 2, 2048, 1024
H, HD = 16, 64
FEA = H * HD
NCORES = 8

DT = 8            # 1024 contraction dim / 128
JT = 16           # key tiles of 128
IB = 4            # query blocks of 512
NP = 2            # head pairs per core
VW = 256          # v storage: [1|pad63|vA(64) | 1|pad63|vB(64)] per s-tile
NSLOT = NP * IB * 8
LAG = 4           # attended runs LAG slots behind its exp
USE_XBAR_V = True

BF16 = mybir.dt.bfloat16
F32 = mybir.dt.float32
AF = mybir.ActivationFunctionType
ALU = mybir.AluOpType

_NC_CACHE = {}

C_SCORES = 470
C_ATT = 880
C_PROJ = 1820
C_VTR = 60 if USE_XBAR_V else 330
C_OUTP = 520
SLOT_NS = 2292    # 2 ACTIVATEs of N=1024
AHEAD = 2000
READY_NB = {0: -1, 1: 2, 2: 4, 3: 4}


def _emit(tc, xT, wk, wq, wv, wo, out):
    nc = tc.nc
    with (
        tc.tile_pool(name="consts", bufs=1) as consts,
        tc.tile_pool(name="stp", bufs=2) as stp,
        tc.tile_pool(name="small", bufs=2) as small,
        tc.tile_pool(name="osbp", bufs=2) as osbp,
        tc.tile_pool(name="ps_sc", bufs=2, space="PSUM") as ps_sc,
        tc.tile_pool(name="ps_att", bufs=2, space="PSUM") as ps_att,
        tc.tile_pool(name="ps_small", bufs=2, space="PSUM") as ps_small,
    ):
        xts = consts.tile([128, DT * S], BF16, tag="xts")
        wks = consts.tile([128, DT * 256], BF16, tag="wks")
        wqs = consts.tile([128, DT * 256], BF16, tag="wqs")
        wvs = consts.tile([128, DT * 256], BF16, tag="wvs")
        wos = consts.tile([128, NP * D], BF16, tag="wos")
        qTs = [consts.tile([128, S], BF16, name=f"qT{p}", tag=f"qT{p}") for p in range(NP)]
        kTs = [consts.tile([128, S], BF16, name=f"kT{p}", tag=f"kT{p}") for p in range(NP)]
        vTs = [consts.tile([128, S], BF16, name=f"vT{p}", tag=f"vT{p}") for p in range(NP)]
        vsbs = [consts.tile([128, JT * VW], BF16, name=f"v{p}", tag=f"v{p}") for p in range(NP)]
        attTs = [consts.tile([128, S], BF16, name=f"attT{p}", tag=f"attT{p}") for p in range(NP)]
        ident = consts.tile([128, 128], BF16, tag="ident")
        dummy = consts.tile([1, 1], F32, tag="dummy")

        # x is packed block-major on the host: [p, nb(4), dt(8), c(512)] so
        # each 512-token block is one fully contiguous 8KB/partition DMA and
        # proj(nb) only waits on block nb's transfer.
        xb = xts.rearrange("p (nb c) -> p nb c", nb=4)
        xTb = xT.rearrange("p (nb c) -> p nb c", nb=4)
        nc.sync.dma_start(out=wks, in_=wk[:, :])
        nc.sync.dma_start(out=xb[:, 0:1, 0:2048], in_=xTb[:, 0:1, 0:2048])
        nc.sync.dma_start(out=xb[:, 0:1, 2048:4096], in_=xTb[:, 0:1, 2048:4096])
        nc.sync.dma_start(out=wqs, in_=wq[:, :])
        nc.sync.dma_start(out=xb[:, 1:2], in_=xTb[:, 1:2])
        nc.sync.dma_start(out=wvs, in_=wv[:, :])
        nc.sync.dma_start(out=xb[:, 2:3], in_=xTb[:, 2:3])
        nc.sync.dma_start(out=xb[:, 3:4], in_=xTb[:, 3:4])
        nc.sync.dma_start(out=wos, in_=wo[:, :])

        # preload the exp table set on ScalarE before the first real ACTIVATE
        nc.vector.memset(dummy, 0.0)
        nc.scalar.activation(out=dummy, in_=dummy, func=AF.Exp, scale=0.125)
        # warm-up burst: ~3.7us of cold matmuls flips the PE clock gate (HAM)
        # to 8/8 right around when x block 0 lands; the two rotating PSUM
        # tiles avoid WAW serialization between the warm matmuls.
        warm = consts.tile([128, 512], BF16, tag="warm")
        nc.vector.memset(warm, 0.0)
        for _ in range(6):
            wps = ps_sc.tile([128, 1024], F32, name="sc", tag="sc")
            nc.tensor.matmul(wps[:, 0:512], lhsT=warm[:, 0:128], rhs=warm,
                             start=True, stop=True)
            nc.tensor.matmul(wps[:, 512:1024], lhsT=warm[:, 0:128], rhs=warm,
                             start=True, stop=True)
        make_identity(nc, ident)

        # ---------------- emitters ----------------
        def emit_proj(ws, p, nb, dst):
            scol = nb * 512
            ps = ps_small.tile([128, 512], F32, name="pss", tag="ps_small")
            for dt in range(DT):
                nc.tensor.matmul(
                    ps,
                    lhsT=ws[:, dt * 256 + p * 128: dt * 256 + (p + 1) * 128],
                    rhs=xts[:, (nb * DT + dt) * 512: (nb * DT + dt + 1) * 512],
                    start=(dt == 0),
                    stop=(dt == DT - 1),
                )
            nc.vector.tensor_copy(out=dst[:, scol:scol + 512], in_=ps)

        def emit_v(p, nb):
            # vT's [f=128, 512-tok] block -> four [s, f] tiles landed as
            # [1|pad63|vA | 1|pad63|vB] via one XBAR transpose DMA per head
            # (the 3D out AP's middle dim is the partition-block index).
            # Ones preset by memset; softmax denominators then fall out of
            # the attended matmul on PSUM partition 0.
            vview = vsbs[p].rearrange("p (st w) -> p st w", w=VW)
            for h in range(2):
                nc.sync.dma_start(
                    out=vview[:, 4 * nb:4 * nb + 4, h * 128 + 64: h * 128 + 128],
                    in_=vTs[p][h * 64:(h + 1) * 64, nb * 512:(nb + 1) * 512],
                    transpose=True,
                )

        st_tiles = {}
        att_ps = {}
        osb_tiles = {}
        sc_pend = {}

        def emit_scores(T):
            p, ib, g = T // 32, (T % 32) // 8, T % 8
            if g == 0:
                st_tiles[(p, ib)] = stp.tile(
                    [128, JT * 1024], BF16, name="stAB", tag="st")
            icol = ib * 512
            tiles = []
            for jt in (2 * g, 2 * g + 1):
                sc = ps_sc.tile([128, 1024], F32, name="sc", tag="sc")
                for h, hsl in ((0, slice(0, 64)), (1, slice(64, 128))):
                    nc.tensor.matmul(
                        sc[:, h * 512:(h + 1) * 512],
                        lhsT=kTs[p][hsl, jt * 128:(jt + 1) * 128],
                        rhs=qTs[p][hsl, icol:icol + 512],
                        start=True,
                        stop=True,
                    )
                tiles.append(sc)
            sc_pend[T] = tiles

        def emit_exp(T):
            p, ib, g = T // 32, (T % 32) // 8, T % 8
            stAB = st_tiles[(p, ib)]
            tiles = sc_pend.pop(T)
            for idx, jt in enumerate((2 * g, 2 * g + 1)):
                nc.scalar.activation(
                    out=stAB[:, jt * 1024:(jt + 1) * 1024],
                    in_=tiles[idx][:, 0:1024],
                    func=AF.Exp,
                    scale=0.125,
                )

        def normalize(p, ib):
            # copy-less: DVE reads the attended PSUM directly for both the
            # reciprocal and the multiply (one PSUM operand per op), halving
            # Vector's normalize work; bcasts run on GpSimd in between.
            # The PSUM banks release at the mult, ~2 super-slots before the
            # next ib's attended needs them.
            icol = ib * 512
            psA, psB = att_ps[(p, ib)]
            rrs = []
            for h, ps in ((0, psA), (1, psB)):
                rrow = small.tile([1, 512], F32, name="rrow", tag="rrow")
                nc.vector.reciprocal_approx_fast(out=rrow, in_=ps[0:1, :])
                rrs.append(rrow)
            rbs = []
            for h in range(2):
                rb = small.tile([128, 512], F32, name="rb", tag="rb")
                nc.gpsimd.partition_broadcast(rb, rrs[h])
                rbs.append(rb)
            for h, ps in ((0, psA), (1, psB)):
                nc.vector.tensor_tensor(
                    out=attTs[p][h * 64:(h + 1) * 64, icol:icol + 512],
                    in0=ps[64:128, :],
                    in1=rbs[h][64:128, :],
                    op=ALU.mult,
                )

        def emit_outproj(p, ib, u, wide=False):
            if u == 0:
                osb_tiles[(p, ib)] = osbp.tile([128, 4096], BF16, name="osb", tag="osb")
            osb = osb_tiles[(p, ib)]
            tt = ib * 4 + u
            if wide:
                ps = ps_sc.tile([128, 1024], F32, name="sc", tag="sc")
                for db in range(2):
                    nc.tensor.matmul(
                        ps[:, db * 512:(db + 1) * 512],
                        lhsT=attTs[p][:, tt * 128:(tt + 1) * 128],
                        rhs=wos[:, p * D + db * 512: p * D + (db + 1) * 512],
                        start=True,
                        stop=True,
                    )
                nc.vector.tensor_copy(
                    out=osb[:, u * 1024:(u + 1) * 1024], in_=ps)
            else:
                for db in range(2):
                    ps = ps_small.tile([128, 512], F32, name="pss", tag="ps_small")
                    nc.tensor.matmul(
                        ps,
                        lhsT=attTs[p][:, tt * 128:(tt + 1) * 128],
                        rhs=wos[:, p * D + db * 512: p * D + (db + 1) * 512],
                        start=True,
                        stop=True,
                    )
                    nc.vector.tensor_copy(
                        out=osb[:, u * 1024 + db * 512: u * 1024 + (db + 1) * 512],
                        in_=ps)
            if wide:
                ov = osb.rearrange("p (uu db c) -> p uu db c", uu=4, db=2)[:, u:u + 1]
                r0 = (p * IB + ib) * 512 + u * 128
                dv = out[r0:r0 + 128, :].rearrange(
                    "(uu prt) (db c) -> prt uu db c", uu=1, db=2)
                nc.scalar.dma_start(out=dv, in_=ov)
            elif u == 3:
                ov = osb.rearrange("p (uu db c) -> p uu db c", uu=4, db=2)
                r0 = (p * IB + ib) * 512
                dv = out[r0:r0 + 512, :].rearrange(
                    "(uu prt) (db c) -> prt uu db c", uu=4, db=2
                )
                nc.sync.dma_start(out=dv, in_=ov)

        # ---------------- conductor ----------------
        units = {}      # name -> [deadline, ready, cost, fn, reqs, done]
        order = []
        spent = [2400.0]

        def add(name, deadline, ready, cost, fn, reqs=()):
            units[name] = [deadline, ready, cost, fn, list(reqs), False]
            order.append(name)

        def force(name):
            u = units.get(name)
            if u is None or u[5]:
                return
            for r in u[4]:
                force(r)
            u[5] = True
            u[3]()
            spent[0] += u[2]

        for p in range(NP):
            base = p * 32
            for nb in range(4):
                dk = base + 2 * nb - 2 if p == 0 else 12 + 4 * nb
                dq = base + 8 * nb - 2 if p == 0 else min(14 + 6 * nb, 30 + 8 * nb - 2)
                dv_ = base + 2 * nb + 2 if p == 0 else 18 + 4 * nb
                add(f"K{p}{nb}", dk, READY_NB[nb], C_PROJ,
                    (lambda p=p, nb=nb: emit_proj(wks, p, nb, kTs[p])))
                add(f"Q{p}{nb}", dq, READY_NB[nb], C_PROJ,
                    (lambda p=p, nb=nb: emit_proj(wqs, p, nb, qTs[p])))
                add(f"VT{p}{nb}", dv_, READY_NB[nb], C_PROJ,
                    (lambda p=p, nb=nb: emit_proj(wvs, p, nb, vTs[p])))
            add(f"MS{p}", base + 2, None, 0,
                (lambda p=p: nc.vector.memset(vsbs[p], 1.0)))
            for nb in range(4):
                add(f"V{p}_{nb}", base + 2 * nb + 3, None, C_VTR,
                    (lambda p=p, nb=nb: emit_v(p, nb)),
                    reqs=[f"VT{p}{nb}", f"MS{p}"])

        def scores_with_reqs(T):
            p, ib, g = T // 32, (T % 32) // 8, T % 8
            force(f"K{p}{g // 2}")
            force(f"Q{p}{ib}")
            emit_scores(T)
            spent[0] += C_SCORES

        def emit_att_group(Ts, slot, wide_op=False):
            # all Ts share (p, ib); heads outer so every matmul in a row
            # shares both its PSUM tile and its rhs SBUF tile (stAB) with
            # its neighbor -- rhs-tile switches cost ~106ns on the PE
            p, ib = Ts[0] // 32, (Ts[0] % 32) // 8
            gs = [T % 8 for T in Ts]
            for g in gs:
                force(f"V{p}_{g // 2}")
            if gs[0] == 0:
                att_ps[(p, ib)] = (
                    ps_att.tile([128, 512], F32, name="attA", tag="att"),
                    ps_att.tile([128, 512], F32, name="attB", tag="att"),
                )
            stAB = st_tiles[(p, ib)]
            psA, psB = att_ps[(p, ib)]
            def att_mm(h, ps, jts):
                for jt in jts:
                    nc.tensor.matmul(
                        ps[0:128, :],
                        lhsT=vsbs[p][:, jt * VW + h * 128:
                                     jt * VW + h * 128 + 128],
                        rhs=stAB[:, jt * 1024 + h * 512:
                                 jt * 1024 + (h + 1) * 512],
                        start=(jt == 0),
                        stop=(jt == JT - 1),
                    )

            if wide_op and gs[-1] == 7:
                # tail: jt-major so the wait for the final EXP sits between
                # whole per-head chains, with warm filler matmuls keeping the
                # PE p-state up through the idle window; then a pipelined
                # V/G normalize and the wide outprojs
                early = [2 * g + d for g in gs[:-1] for d in (0, 1)]
                late = [2 * gs[-1], 2 * gs[-1] + 1]
                att_mm(0, psA, early)
                att_mm(1, psB, early)
                for _ in range(8):
                    wps = ps_small.tile([128, 512], F32, name="pss",
                                        tag="ps_small")
                    nc.tensor.matmul(wps, lhsT=warm[:, 0:128], rhs=warm,
                                     start=True, stop=True)
                att_mm(0, psA, late)
                att_mm(1, psB, late)
                normalize(p, ib)
                # warm filler executes during the normalize V/G chain so the
                # final outprojs run at full p-state
                for _ in range(14):
                    wps = ps_small.tile([128, 512], F32, name="pss",
                                        tag="ps_small")
                    nc.tensor.matmul(wps, lhsT=warm[:, 0:128], rhs=warm,
                                     start=True, stop=True)
                for u in range(4):
                    emit_outproj(p, ib, u, wide=True)
            else:
                for h, ps in ((0, psA), (1, psB)):
                    att_mm(h, ps, [jt for g in gs for jt in (2 * g, 2 * g + 1)])
            spent[0] += C_ATT * len(Ts)
            if gs[-1] == 7 and not wide_op:
                normalize(p, ib)
                for u in range(4):
                    add(f"OP{p}{ib}{u}", slot + 3 + 2 * u, None, C_OUTP,
                        (lambda p=p, ib=ib, u=u: emit_outproj(p, ib, u)),
                        reqs=([f"OP{p}{ib}{u - 1}"] if u else []))

        def run_filler(t, budget):
            for name in sorted(
                (n for n in order if not units[n][5]),
                key=lambda n: (units[n][0], n),
            ):
                u = units[name]
                if u[5]:
                    continue
                if u[1] is not None and u[1] > t:
                    continue
                if u[0] <= t + 1 or spent[0] + u[2] <= budget:
                    force(name)

        # prefix: scores two slots ahead (double-buffered sc PSUM pair)
        force("K00")
        force("Q00")
        scores_with_reqs(0)
        scores_with_reqs(1)

        # super-slots: two exp slots per iteration; attended grouped across
        # both slots, scores for t0+2/t0+3 emitted back-to-back -> half the
        # rhs-tile switches on the PE
        for s in range(NSLOT // 2):
            t0, t1 = 2 * s, 2 * s + 1
            emit_exp(t0)
            emit_exp(t1)
            if s == 30:
                # clear every leftover unit now so Vector/PE queues are empty
                # for the tail's critical chain
                for name in order:
                    force(name)
                groups = [[56, 57], [58, 59]]
            elif s == 31:
                groups = [[60, 61], [62, 63]]
            else:
                groups = []
                for T in (t0 - LAG, t0 - LAG + 1):
                    if not (0 <= T < NSLOT - 8):
                        continue
                    if groups and T % 8 != 0:
                        groups[-1].append(T)
                    else:
                        groups.append([T])
            # a g0-leading group allocates fresh att PSUM tiles (waits on the
            # previous ib's normalize mults); emit scores first there so the
            # wait never sits at the PE queue head
            g0_first = bool(groups) and groups[0][0] % 8 == 0 and s != 31
            if g0_first and t0 + 2 < NSLOT:
                scores_with_reqs(t0 + 2)
            for G in groups:
                emit_att_group(G, t1, wide_op=(s == 31 and 63 in G))
            if g0_first:
                if t0 + 3 < NSLOT:
                    scores_with_reqs(t0 + 3)
            elif s < 2:
                # early phase: sandwich each scores pair between filler so
                # its wait on the Scalar EXP never idles the PE queue head
                if t0 + 2 < NSLOT:
                    run_filler(t0, spent[0] + C_PROJ)
                    scores_with_reqs(t0 + 2)
                if t0 + 3 < NSLOT:
                    run_filler(t0, spent[0] + C_PROJ)
                    scores_with_reqs(t0 + 3)
            else:
                if t0 + 2 < NSLOT:
                    scores_with_reqs(t0 + 2)
                if t0 + 3 < NSLOT:
                    scores_with_reqs(t0 + 3)
            run_filler(t1, (t1 + 1) * SLOT_NS + AHEAD)

        for name in order:
            force(name)


def build_nc():
    if "nc" in _NC_CACHE:
        return _NC_CACHE["nc"]
    nc = bacc.Bacc("TRN2", debug=False, num_devices=NCORES)
    xT = nc.dram_tensor("xT", [128, DT * S], BF16, kind="ExternalInput").ap()
    wk = nc.dram_tensor("wk", [128, DT * 256], BF16, kind="ExternalInput").ap()
    wq = nc.dram_tensor("wq", [128, DT * 256], BF16, kind="ExternalInput").ap()
    wv = nc.dram_tensor("wv", [128, DT * 256], BF16, kind="ExternalInput").ap()
    wo = nc.dram_tensor("wo", [128, NP * D], BF16, kind="ExternalInput").ap()
    out = nc.dram_tensor("out", [NP * S, D], BF16, kind="ExternalOutput").ap()
    with tile.TileContext(nc) as tc:
        _emit(tc, xT, wk, wq, wv, wo, out)
    nc.compile()
    _NC_CACHE["nc"] = nc
    return nc


def _pack_w(slices):
    """slices: per-pair [128 feat, D] arrays -> [128, DT*256] bf16 with per-dt
    free-axis layout [pair0 128 | pair1 128] and the D contraction on partitions."""
    bf = ml_dtypes.bfloat16
    parts = [np.ascontiguousarray(a.T).reshape(DT, 128, 128) for a in slices]
    pack = np.stack(parts, axis=2)  # [dt, 128drow, pair, 128f]
    return np.ascontiguousarray(
        pack.transpose(1, 0, 2, 3).reshape(128, DT * 256)
    ).astype(bf)


def make_in_maps(x, qkv_w, out_w):
    bf = ml_dtypes.bfloat16
    maps = []
    for c in range(NCORES):
        b, quad = c // 4, c % 4
        xT = np.ascontiguousarray(x[b].T)  # [1024, 2048]
        # block-major: [p, nb(4), dt(8), c(512)] so each 512-token block is
        # contiguous per partition (one fast DMA per block)
        xTd = np.ascontiguousarray(
            xT.reshape(DT, 128, 4, 512).transpose(1, 2, 0, 3).reshape(128, DT * S)
        ).astype(bf)
        wkp, wqp, wvp, wop = [], [], [], []
        for p in range(NP):
            hA, hB = 4 * quad + 2 * p, 4 * quad + 2 * p + 1
            wqp.append(np.concatenate(
                [qkv_w[hA * 192: hA * 192 + 64], qkv_w[hB * 192: hB * 192 + 64]], 0))
            wkp.append(np.concatenate(
                [qkv_w[hA * 192 + 64: hA * 192 + 128],
                 qkv_w[hB * 192 + 64: hB * 192 + 128]], 0))
            wvp.append(np.concatenate(
                [qkv_w[hA * 192 + 128: hA * 192 + 192],
                 qkv_w[hB * 192 + 128: hB * 192 + 192]], 0))
            wop.append(np.concatenate(
                [out_w[:, hA * 64: hA * 64 + 64], out_w[:, hB * 64: hB * 64 + 64]],
                1).T)  # [128 feat, 1024 D]
        maps.append({
            "xT": xTd,
            "wk": _pack_w(wkp),
            "wq": _pack_w(wqp),
            "wv": _pack_w(wvp),
            "wo": np.ascontiguousarray(np.concatenate(wop, 1)).astype(bf),
        })
    return maps


def kernel(x, qkv_w, out_w, out_b, _run_kwargs=None):
    x = np.asarray(x, dtype=np.float32)
    qkv_w = np.asarray(qkv_w, dtype=np.float32)
    out_w = np.asarray(out_w, dtype=np.float32)
    out_b = np.asarray(out_b, dtype=np.float32)

    nc = build_nc()
    in_maps = make_in_maps(x, qkv_w, out_w)
    res = run_bass_kernel_spmd(
        nc, in_maps, list(range(NCORES)), **(_run_kwargs or {})
    )
    total = np.zeros((B, S, D), np.float32)
    for c in range(NCORES):
        b = c // 4
        r = np.asarray(res.results[c]["out"]).astype(np.float32)
        total[b] += r[0:S] + r[S:2 * S]
    total += out_b[None, None, :]
    if _run_kwargs:
        kernel.last_result = res
    return total



# revision 22
# speedup vs baseline: 1.0195x; 1.0195x over previous
"""Trainium2 Bass kernel for multi-head attention (B=2, S=2048, D=1024, H=16, HD=64).

Sharding: batch x head-quad. Core c owns batch c//4 and heads 4*(c%4)..4*(c%4)+3,
organized as two head PAIRS (p=0,1), each pair's two heads on partition halves
0:64 / 64:128 so the K=64 score matmuls run concurrently via PE row-group tiling.

Emission is a 64-slot conductor (slot t -> (pair, query-block ib, key-group g)):
  - ScalarE: 2 exp ACTIVATEs per slot (one per key tile, N=1024 covering BOTH
    heads side by side so the head A/B score matmuls share one PSUM tile and
    become ready together -> the PE runs them concurrently via row groups).
  - PE: scores for slot t+1 (PSUM ping-pong against the ACT reads), the attended
    matmuls of slot t-4 ([1|pad63|v] stationary emits softmax denominators on
    PSUM partition 0 for free), then deadline-scheduled filler (qkv projection
    blocks, output projection) against a running ns budget. Hard data-order
    requirements are enforced by force-running prerequisite units.
  - v transposes go through the DMA XBAR (SBUF->SBUF, transpose=True), not the PE.
  - last ib's attended is compressed into slots 60-63 (2 groups/slot) and its
    output projection uses wide PSUM tiles so the tail stays short.
Host packs weights into exact SBUF layouts; outputs are written bf16 and summed
on the host.
"""

import numpy as np
import ml_dtypes

import concourse.bacc as bacc
import concourse.tile as tile
import concourse.mybir as mybir
from concourse.bass_utils import run_bass_kernel_spmd
from concourse.masks import make_identity

B, S, D = 2, 2048, 1024
H, HD = 16, 64
FEA = H * HD
NCORES = 8

DT = 8            # 1024 contraction dim / 128
JT = 16           # key tiles of 128
IB = 4            # query blocks of 512
NP = 2            # head pairs per core
VW = 256          # v storage: [1|pad63|vA(64) | 1|pad63|vB(64)] per s-tile
NSLOT = NP * IB * 8
LAG = 4           # attended runs LAG slots behind its exp
USE_XBAR_V = True

BF16 = mybir.dt.bfloat16
F32 = mybir.dt.float32
AF = mybir.ActivationFunctionType
ALU = mybir.AluOpType

_NC_CACHE = {}

C_SCORES = 470
C_ATT = 880
C_PROJ = 1820
C_VTR = 60 if USE_XBAR_V else 330
C_OUTP = 520
SLOT_NS = 2292    # 2 ACTIVATEs of N=1024
AHEAD = 2000
READY_NB = {0: -1, 1: 2, 2: 4, 3: 4}


def _emit(tc, xT, wk, wq, wv, wo, out):
    nc = tc.nc
    with (
        tc.tile_pool(name="consts", bufs=1) as consts,
        tc.tile_pool(name="stp", bufs=2) as stp,
        tc.tile_pool(name="small", bufs=2) as small,
        tc.tile_pool(name="osbp", bufs=2) as osbp,
        tc.tile_pool(name="ps_sc", bufs=2, space="PSUM") as ps_sc,
        tc.tile_pool(name="ps_att", bufs=2, space="PSUM") as ps_att,
        tc.tile_pool(name="ps_small", bufs=2, space="PSUM") as ps_small,
    ):
        xts = consts.tile([128, DT * S], BF16, tag="xts")
        wks = consts.tile([128, DT * 256], BF16, tag="wks")
        wqs = consts.tile([128, DT * 256], BF16, tag="wqs")
        wvs = consts.tile([128, DT * 256], BF16, tag="wvs")
        wos = consts.tile([128, NP * D], BF16, tag="wos")
        qTs = [consts.tile([128, S], BF16, name=f"qT{p}", tag=f"qT{p}") for p in range(NP)]
        kTs = [consts.tile([128, S], BF16, name=f"kT{p}", tag=f"kT{p}") for p in range(NP)]
        vTs = [consts.tile([128, S], BF16, name=f"vT{p}", tag=f"vT{p}") for p in range(NP)]
        vsbs = [consts.tile([128, JT * VW], BF16, name=f"v{p}", tag=f"v{p}") for p in range(NP)]
        attTs = [consts.tile([128, S], BF16, name=f"attT{p}", tag=f"attT{p}") for p in range(NP)]
        ident = consts.tile([128, 128], BF16, tag="ident")
        dummy = consts.tile([1, 1], F32, tag="dummy")

        # x is packed block-major on the host: [p, nb(4), dt(8), c(512)] so
        # each 512-token block is one fully contiguous 8KB/partition DMA and
        # proj(nb) only waits on block nb's transfer.
        xb = xts.rearrange("p (nb c) -> p nb c", nb=4)
        xTb = xT.rearrange("p (nb c) -> p nb c", nb=4)
        nc.sync.dma_start(out=wks, in_=wk[:, :])
        nc.sync.dma_start(out=xb[:, 0:1, 0:2048], in_=xTb[:, 0:1, 0:2048])
        nc.sync.dma_start(out=xb[:, 0:1, 2048:4096], in_=xTb[:, 0:1, 2048:4096])
        nc.sync.dma_start(out=wqs, in_=wq[:, :])
        nc.sync.dma_start(out=xb[:, 1:2], in_=xTb[:, 1:2])
        nc.sync.dma_start(out=wvs, in_=wv[:, :])
        nc.sync.dma_start(out=xb[:, 2:3], in_=xTb[:, 2:3])
        nc.sync.dma_start(out=xb[:, 3:4], in_=xTb[:, 3:4])
        nc.sync.dma_start(out=wos, in_=wo[:, :])

        # preload the exp table set on ScalarE before the first real ACTIVATE
        nc.vector.memset(dummy, 0.0)
        nc.scalar.activation(out=dummy, in_=dummy, func=AF.Exp, scale=0.125)
        # warm-up burst: ~3.7us of cold matmuls flips the PE clock gate (HAM)
        # to 8/8 right around when x block 0 lands; the two rotating PSUM
        # tiles avoid WAW serialization between the warm matmuls.
        warm = consts.tile([128, 512], BF16, tag="warm")
        nc.vector.memset(warm, 0.0)
        for _ in range(6):
            wps = ps_sc.tile([128, 1024], F32, name="sc", tag="sc")
            nc.tensor.matmul(wps[:, 0:512], lhsT=warm[:, 0:128], rhs=warm,
                             start=True, stop=True)
            nc.tensor.matmul(wps[:, 512:1024], lhsT=warm[:, 0:128], rhs=warm,
                             start=True, stop=True)
        make_identity(nc, ident)

        # ---------------- emitters ----------------
        def emit_proj(ws, p, nb, dst):
            scol = nb * 512
            ps = ps_small.tile([128, 512], F32, name="pss", tag="ps_small")
            for dt in range(DT):
                nc.tensor.matmul(
                    ps,
                    lhsT=ws[:, dt * 256 + p * 128: dt * 256 + (p + 1) * 128],
                    rhs=xts[:, (nb * DT + dt) * 512: (nb * DT + dt + 1) * 512],
                    start=(dt == 0),
                    stop=(dt == DT - 1),
                )
            nc.vector.tensor_copy(out=dst[:, scol:scol + 512], in_=ps)

        def emit_v(p, nb):
            # vT's [f=128, 512-tok] block -> four [s, f] tiles landed as
            # [1|pad63|vA | 1|pad63|vB] via one XBAR transpose DMA per head
            # (the 3D out AP's middle dim is the partition-block index).
            # Ones preset by memset; softmax denominators then fall out of
            # the attended matmul on PSUM partition 0.
            vview = vsbs[p].rearrange("p (st w) -> p st w", w=VW)
            for h in range(2):
                nc.sync.dma_start(
                    out=vview[:, 4 * nb:4 * nb + 4, h * 128 + 64: h * 128 + 128],
                    in_=vTs[p][h * 64:(h + 1) * 64, nb * 512:(nb + 1) * 512],
                    transpose=True,
                )

        st_tiles = {}
        att_ps = {}
        osb_tiles = {}
        sc_pend = {}

        def emit_scores(T):
            p, ib, g = T // 32, (T % 32) // 8, T % 8
            if g == 0:
                st_tiles[(p, ib)] = stp.tile(
                    [128, JT * 1024], BF16, name="stAB", tag="st")
            icol = ib * 512
            tiles = []
            for jt in (2 * g, 2 * g + 1):
                sc = ps_sc.tile([128, 1024], F32, name="sc", tag="sc")
                for h, hsl in ((0, slice(0, 64)), (1, slice(64, 128))):
                    nc.tensor.matmul(
                        sc[:, h * 512:(h + 1) * 512],
                        lhsT=kTs[p][hsl, jt * 128:(jt + 1) * 128],
                        rhs=qTs[p][hsl, icol:icol + 512],
                        start=True,
                        stop=True,
                    )
                tiles.append(sc)
            sc_pend[T] = tiles

        def emit_exp(T):
            p, ib, g = T // 32, (T % 32) // 8, T % 8
            stAB = st_tiles[(p, ib)]
            tiles = sc_pend.pop(T)
            for idx, jt in enumerate((2 * g, 2 * g + 1)):
                nc.scalar.activation(
                    out=stAB[:, jt * 1024:(jt + 1) * 1024],
                    in_=tiles[idx][:, 0:1024],
                    func=AF.Exp,
                    scale=0.125,
                )

        def normalize(p, ib):
            # copy-less: DVE reads the attended PSUM directly for both the
            # reciprocal and the multiply (one PSUM operand per op), halving
            # Vector's normalize work; bcasts run on GpSimd in between.
            # The PSUM banks release at the mult, ~2 super-slots before the
            # next ib's attended needs them.
            icol = ib * 512
            psA, psB = att_ps[(p, ib)]
            rrs = []
            for h, ps in ((0, psA), (1, psB)):
                rrow = small.tile([1, 512], F32, name="rrow", tag="rrow")
                nc.vector.reciprocal_approx_fast(out=rrow, in_=ps[0:1, :])
                rrs.append(rrow)
            rbs = []
            for h in range(2):
                rb = small.tile([128, 512], F32, name="rb", tag="rb")
                nc.gpsimd.partition_broadcast(rb, rrs[h])
                rbs.append(rb)
            for h, ps in ((0, psA), (1, psB)):
                nc.vector.tensor_tensor(
                    out=attTs[p][h * 64:(h + 1) * 64, icol:icol + 512],
                    in0=ps[64:128, :],
                    in1=rbs[h][64:128, :],
                    op=ALU.mult,
                )

        def emit_outproj(p, ib, u, wide=False):
            if u == 0:
                osb_tiles[(p, ib)] = osbp.tile([128, 4096], BF16, name="osb", tag="osb")
            osb = osb_tiles[(p, ib)]
            tt = ib * 4 + u
            if wide:
                ps = ps_sc.tile([128, 1024], F32, name="sc", tag="sc")
                for db in range(2):
                    nc.tensor.matmul(
                        ps[:, db * 512:(db + 1) * 512],
                        lhsT=attTs[p][:, tt * 128:(tt + 1) * 128],
                        rhs=wos[:, p * D + db * 512: p * D + (db + 1) * 512],
                        start=True,
                        stop=True,
                    )
                nc.vector.tensor_copy(
                    out=osb[:, u * 1024:(u + 1) * 1024], in_=ps)
            else:
                for db in range(2):
                    ps = ps_small.tile([128, 512], F32, name="pss", tag="ps_small")
                    nc.tensor.matmul(
                        ps,
                        lhsT=attTs[p][:, tt * 128:(tt + 1) * 128],
                        rhs=wos[:, p * D + db * 512: p * D + (db + 1) * 512],
                        start=True,
                        stop=True,
                    )
                    nc.vector.tensor_copy(
                        out=osb[:, u * 1024 + db * 512: u * 1024 + (db + 1) * 512],
                        in_=ps)
            if wide:
                ov = osb.rearrange("p (uu db c) -> p uu db c", uu=4, db=2)[:, u:u + 1]
                r0 = (p * IB + ib) * 512 + u * 128
                dv = out[r0:r0 + 128, :].rearrange(
                    "(uu prt) (db c) -> prt uu db c", uu=1, db=2)
                nc.scalar.dma_start(out=dv, in_=ov)
            elif u == 3:
                ov = osb.rearrange("p (uu db c) -> p uu db c", uu=4, db=2)
                r0 = (p * IB + ib) * 512
                dv = out[r0:r0 + 512, :].rearrange(
                    "(uu prt) (db c) -> prt uu db c", uu=4, db=2
                )
                nc.sync.dma_start(out=dv, in_=ov)

        # ---------------- conductor ----------------
        units = {}      # name -> [deadline, ready, cost, fn, reqs, done]
        order = []
        spent = [2400.0]

        def add(name, deadline, ready, cost, fn, reqs=()):
            units[name] = [deadline, ready, cost, fn, list(reqs), False]
            order.append(name)

        def force(name):
            u = units.get(name)
            if u is None or u[5]:
                return
            for r in u[4]:
                force(r)
            u[5] = True
            u[3]()
            spent[0] += u[2]

        for p in range(NP):
            base = p * 32
            for nb in range(4):
                dk = base + 2 * nb - 2 if p == 0 else 12 + 4 * nb
                dq = base + 8 * nb - 2 if p == 0 else min(14 + 6 * nb, 30 + 8 * nb - 2)
                dv_ = base + 2 * nb + 2 if p == 0 else 18 + 4 * nb
                add(f"K{p}{nb}", dk, READY_NB[nb], C_PROJ,
                    (lambda p=p, nb=nb: emit_proj(wks, p, nb, kTs[p])))
                add(f"Q{p}{nb}", dq, READY_NB[nb], C_PROJ,
                    (lambda p=p, nb=nb: emit_proj(wqs, p, nb, qTs[p])))
                add(f"VT{p}{nb}", dv_, READY_NB[nb], C_PROJ,
                    (lambda p=p, nb=nb: emit_proj(wvs, p, nb, vTs[p])))
            add(f"MS{p}", base + 2, None, 0,
                (lambda p=p: nc.vector.memset(vsbs[p], 1.0)))
            for nb in range(4):
                add(f"V{p}_{nb}", base + 2 * nb + 3, None, C_VTR,
                    (lambda p=p, nb=nb: emit_v(p, nb)),
                    reqs=[f"VT{p}{nb}", f"MS{p}"])

        def scores_with_reqs(T):
            p, ib, g = T // 32, (T % 32) // 8, T % 8
            force(f"K{p}{g // 2}")
            force(f"Q{p}{ib}")
            emit_scores(T)
            spent[0] += C_SCORES

        def emit_att_group(Ts, slot, wide_op=False):
            # all Ts share (p, ib); heads outer so every matmul in a row
            # shares both its PSUM tile and its rhs SBUF tile (stAB) with
            # its neighbor -- rhs-tile switches cost ~106ns on the PE
            p, ib = Ts[0] // 32, (Ts[0] % 32) // 8
            gs = [T % 8 for T in Ts]
            for g in gs:
                force(f"V{p}_{g // 2}")
            if gs[0] == 0:
                att_ps[(p, ib)] = (
                    ps_att.tile([128, 512], F32, name="attA", tag="att"),
                    ps_att.tile([128, 512], F32, name="attB", tag="att"),
                )
            stAB = st_tiles[(p, ib)]
            psA, psB = att_ps[(p, ib)]
            def att_mm(h, ps, jts):
                for jt in jts:
                    nc.tensor.matmul(
                        ps[0:128, :],
                        lhsT=vsbs[p][:, jt * VW + h * 128:
                                     jt * VW + h * 128 + 128],
                        rhs=stAB[:, jt * 1024 + h * 512:
                                 jt * 1024 + (h + 1) * 512],
                        start=(jt == 0),
                        stop=(jt == JT - 1),
                    )

            if wide_op and gs[-1] == 7:
                # tail: jt-major so the wait for the final EXP sits between
                # whole per-head chains, with warm filler matmuls keeping the
                # PE p-state up through the idle window; then a pipelined
                # V/G normalize and the wide outprojs
                early = [2 * g + d for g in gs[:-1] for d in (0, 1)]
                late = [2 * gs[-1], 2 * gs[-1] + 1]
                att_mm(0, psA, early)
                att_mm(1, psB, early)
                for _ in range(4):
                    wps = ps_small.tile([128, 512], F32, name="pss",
                                        tag="ps_small")
                    nc.tensor.matmul(wps, lhsT=warm[:, 0:128], rhs=warm,
                                     start=True, stop=True)
                att_mm(0, psA, late)
                att_mm(1, psB, late)
                normalize(p, ib)
                # warm filler executes during the normalize V/G chain so the
                # final outprojs run at full p-state
                for _ in range(10):
                    wps = ps_small.tile([128, 512], F32, name="pss",
                                        tag="ps_small")
                    nc.tensor.matmul(wps, lhsT=warm[:, 0:128], rhs=warm,
                                     start=True, stop=True)
                for u in range(4):
                    emit_outproj(p, ib, u, wide=True)
            else:
                for h, ps in ((0, psA), (1, psB)):
                    att_mm(h, ps, [jt for g in gs for jt in (2 * g, 2 * g + 1)])
            spent[0] += C_ATT * len(Ts)
            if gs[-1] == 7 and not wide_op:
                normalize(p, ib)
                for u in range(4):
                    add(f"OP{p}{ib}{u}", slot + 3 + 2 * u, None, C_OUTP,
                        (lambda p=p, ib=ib, u=u: emit_outproj(p, ib, u)),
                        reqs=([f"OP{p}{ib}{u - 1}"] if u else []))

        def run_filler(t, budget):
            for name in sorted(
                (n for n in order if not units[n][5]),
                key=lambda n: (units[n][0], n),
            ):
                u = units[name]
                if u[5]:
                    continue
                if u[1] is not None and u[1] > t:
                    continue
                if u[0] <= t + 1 or spent[0] + u[2] <= budget:
                    force(name)

        # prefix: scores two slots ahead (double-buffered sc PSUM pair)
        force("K00")
        force("Q00")
        scores_with_reqs(0)
        scores_with_reqs(1)

        # super-slots: two exp slots per iteration; attended grouped across
        # both slots, scores for t0+2/t0+3 emitted back-to-back -> half the
        # rhs-tile switches on the PE
        for s in range(NSLOT // 2):
            t0, t1 = 2 * s, 2 * s + 1
            emit_exp(t0)
            emit_exp(t1)
            if s == 30:
                # clear every leftover unit now so Vector/PE queues are empty
                # for the tail's critical chain
                for name in order:
                    force(name)
                groups = [[56, 57], [58, 59]]
            elif s == 31:
                groups = [[60, 61], [62, 63]]
            else:
                groups = []
                for T in (t0 - LAG, t0 - LAG + 1):
                    if not (0 <= T < NSLOT - 8):
                        continue
                    if groups and T % 8 != 0:
                        groups[-1].append(T)
                    else:
                        groups.append([T])
            for G in groups:
                emit_att_group(G, t1, wide_op=(s == 31 and 63 in G))
            if s < 2:
                # early phase: sandwich each scores pair between filler so
                # its wait on the Scalar EXP never idles the PE queue head
                if t0 + 2 < NSLOT:
                    run_filler(t0, spent[0] + C_PROJ)
                    scores_with_reqs(t0 + 2)
                if t0 + 3 < NSLOT:
                    run_filler(t0, spent[0] + C_PROJ)
                    scores_with_reqs(t0 + 3)
            else:
                if t0 + 2 < NSLOT:
                    scores_with_reqs(t0 + 2)
                if t0 + 3 < NSLOT:
                    scores_with_reqs(t0 + 3)
            run_filler(t1, (t1 + 1) * SLOT_NS + AHEAD)

        for name in order:
            force(name)


def build_nc():
    if "nc" in _NC_CACHE:
        return _NC_CACHE["nc"]
    nc = bacc.Bacc("TRN2", debug=False, num_devices=NCORES)
    xT = nc.dram_tensor("xT", [128, DT * S], BF16, kind="ExternalInput").ap()
    wk = nc.dram_tensor("wk", [128, DT * 256], BF16, kind="ExternalInput").ap()
    wq = nc.dram_tensor("wq", [128, DT * 256], BF16, kind="ExternalInput").ap()
    wv = nc.dram_tensor("wv", [128, DT * 256], BF16, kind="ExternalInput").ap()
    wo = nc.dram_tensor("wo", [128, NP * D], BF16, kind="ExternalInput").ap()
    out = nc.dram_tensor("out", [NP * S, D], BF16, kind="ExternalOutput").ap()
    with tile.TileContext(nc) as tc:
        _emit(tc, xT, wk, wq, wv, wo, out)
    nc.compile()
    _NC_CACHE["nc"] = nc
    return nc


def _pack_w(slices):
    """slices: per-pair [128 feat, D] arrays -> [128, DT*256] bf16 with per-dt
    free-axis layout [pair0 128 | pair1 128] and the D contraction on partitions."""
    bf = ml_dtypes.bfloat16
    parts = [np.ascontiguousarray(a.T).reshape(DT, 128, 128) for a in slices]
    pack = np.stack(parts, axis=2)  # [dt, 128drow, pair, 128f]
    return np.ascontiguousarray(
        pack.transpose(1, 0, 2, 3).reshape(128, DT * 256)
    ).astype(bf)


def make_in_maps(x, qkv_w, out_w):
    bf = ml_dtypes.bfloat16
    maps = []
    for c in range(NCORES):
        b, quad = c // 4, c % 4
        xT = np.ascontiguousarray(x[b].T)  # [1024, 2048]
        # block-major: [p, nb(4), dt(8), c(512)] so each 512-token block is
        # contiguous per partition (one fast DMA per block)
        xTd = np.ascontiguousarray(
            xT.reshape(DT, 128, 4, 512).transpose(1, 2, 0, 3).reshape(128, DT * S)
        ).astype(bf)
        wkp, wqp, wvp, wop = [], [], [], []
        for p in range(NP):
            hA, hB = 4 * quad + 2 * p, 4 * quad + 2 * p + 1
            wqp.append(np.concatenate(
                [qkv_w[hA * 192: hA * 192 + 64], qkv_w[hB * 192: hB * 192 + 64]], 0))
            wkp.append(np.concatenate(
                [qkv_w[hA * 192 + 64: hA * 192 + 128],
                 qkv_w[hB * 192 + 64: hB * 192 + 128]], 0))
            wvp.append(np.concatenate(
                [qkv_w[hA * 192 + 128: hA * 192 + 192],
                 qkv_w[hB * 192 + 128: hB * 192 + 192]], 0))
            wop.append(np.concatenate(
                [out_w[:, hA * 64: hA * 64 + 64], out_w[:, hB * 64: hB * 64 + 64]],
                1).T)  # [128 feat, 1024 D]
        maps.append({
            "xT": xTd,
            "wk": _pack_w(wkp),
            "wq": _pack_w(wqp),
            "wv": _pack_w(wvp),
            "wo": np.ascontiguousarray(np.concatenate(wop, 1)).astype(bf),
        })
    return maps


def kernel(x, qkv_w, out_w, out_b, _run_kwargs=None):
    x = np.asarray(x, dtype=np.float32)
    qkv_w = np.asarray(qkv_w, dtype=np.float32)
    out_w = np.asarray(out_w, dtype=np.float32)
    out_b = np.asarray(out_b, dtype=np.float32)

    nc = build_nc()
    in_maps = make_in_maps(x, qkv_w, out_w)
    res = run_bass_kernel_spmd(
        nc, in_maps, list(range(NCORES)), **(_run_kwargs or {})
    )
    total = np.zeros((B, S, D), np.float32)
    for c in range(NCORES):
        b = c // 4
        r = np.asarray(res.results[c]["out"]).astype(np.float32)
        total[b] += r[0:S] + r[S:2 * S]
    total += out_b[None, None, :]
    if _run_kwargs:
        kernel.last_result = res
    return total

